# revision 42
# baseline (speedup 1.0000x reference)
"""Trainium2 Bass kernel for the pairwise-KL contrastive loss (nn_KL_Loss).

Reference math (N=512, D=128, 2N=1024):
    mu  = concat(p1_loc, p2_loc)     [2N, D]
    var = concat(p1_scale, p2_scale) [2N, D]
    kld[i,j] = 0.5 * sum_d( lv[j]-lv[i]-1 + ((mu[i]-mu[j])^2 + var[i])/var[j] )
    sim = where(diag, -9e6, kld) * T          (T = 0.01)
    loss = mean_i( sim[i, (i+N)%2N] - logsumexp_j sim[i,:] )

Kernel decomposition (one 128-row block per core):
    2*kld[i,j] = R[i,j] - L[i] - D,  where
    R[i,j] = sum_d A[i,d]*iv[j,d] + mu[i,d]*muiv'[j,d]
             + sum_d (mu^2*iv)[j,d] + sum_d lv[j,d]
    (A = mu^2 + var, iv = 1/var, lv = log var, muiv' = -2*mu*iv,
     L[i] = sum_d lv[i,d])
    -> 4 TensorE matmuls (K = D = 128) accumulated in PSUM per column group.

    The per-row shift -c*(L[i]+D) cancels in sim_pos - logsumexp, so with
    c = 0.5*T:   loss_i = c*R[i,pos] - log( sum_j exp(c*R[i,j]) - exp(c*(L[i]+D)) )
    The subtracted term removes the diagonal (self) entry exactly
    (R[i,i] = L[i]+D).  sim values are O(1) (max ~2.7) so fp32 sum-of-exps
    is stable without max-subtraction.

Layout strategy (all data prep on HOST, which the contract allows --
sharding/gather happen inside kernel()):
  * Inputs pre-TRANSPOSED to [D, 2N] = [128, 1024]; mu cast to BF16 and
    var to FP8-E4M3 (measured loss rel err 1.4e-5, far under the 2e-2
    gate -- the per-column quantization error largely cancels between
    the positive-pair term and the logsumexp under the row mean).
  * Per core c the columns are rotated by -128c and permuted to
    [own(0:128) | pos-block | rest], so every core runs the identical
    program: self-pairs are the diagonal of columns 0:128 and positive
    pairs the diagonal of columns 128:256 (both in PSUM group 1).
  * Input ships as TWO packed DRAM tensors, one per 512-column half:
    [mu_half bf16 | var_half fp8] = [128, 1536] bytes = 192KB, viewed on
    SBUF through bitcast slices of one uint8 tile.  One dma_start per
    HWDGE queue (sync carries half 0, scalar half 1): a compute half
    becomes ready with ONE semaphore, and there is no second-chunk issue
    serialization on either queue.  (Only sync+scalar have HWDGE; the
    gpsimd SWDGE queue delivers its semaphore ~2.5us late -- avoided.)
  * The reciprocal bit-trick and ACT Ln run directly on the FP8 var
    tiles: DVE/ACT convert operands to fp32 in-pipeline (probe-verified:
    recip-on-fp8 0.3% max err = bf16 output rounding; Ln exact).  FP8
    stays OFF the 2x-rate DVE muls (mu is bf16; a 1-byte dtype would
    drop tensor_tensor to 1x).
  * Output is a [2,1] per-core [sum(C*pos_i), sum(ln S_i)] (PE matmul
    with a ones column contracts the 128 partitions, single_packet DMA);
    the host takes the difference and divides by 2N.

Perf notes (from ntff profiles):
  * Single HWDGE queue streams only ~125-170 GB/s; two run ~300 GB/s
    aggregate.  First packet lands ~0.8us (sync) / ~1.8us (scalar) after
    the issue instruction retires; completion semaphore trails the last
    packet by ~0.35-0.6us.
  * DVE tensor_tensor muls run at the 2x 16-bit rate (~420ns per
    [128,512]); scalar_tensor_tensor measured 1x (~680ns), so the -2 and
    mu^2 foldings live on [128,128] own-block stationaries.  Per-op DVE
    init overhead is ~240ns, but merging ops across halves loses more
    to the exp1->exp2 ACT serialization than it saves (computed).
  * Own-block mu^2 and +var run on GPSIMD tensor_tensor (~430ns each;
    gpsimd tensor_SCALAR costs ~2us).  The -2*mu tensor_scalar stays on
    DVE (pipelines into recip0's shadow, ~110ns): a gpsimd version
    (memset -2 tile) was INTERMITTENTLY WRONG (rel err 0.25 on ~1/3
    runs; gpsimd-write vs PE-LDWEIGHTS race), and an ACT Copy(scale=-2)
    version measured +600ns (disrupted the LN2/exp schedule).
  * The DVE queue order is pinned with tile_wait_until stamps; the Tile
    scheduler's sim otherwise hoists recip1 between recip0 and muiv0,
    delaying R1-close -> exp1 by ~700ns on hardware.
  * Positive-pair gather is ONE custom-DVE tensor_tensor_reduce
    (diag-mask multiply, scale=C, accumulate); the native
    InstTensorTensorReduce opcode hard-faults this firmware.
  * PE DVFS: full matmul clock arrives ~5-6us after sustained PE
    activity begins; the warm-up dummies start immediately behind one
    gpsimd memset so critical matmuls (~376ns) miss the slow-clock
    window (630ns).
  * Rejected by measurement: 4-way quarter-interleaved DMA (+0.2..1.5us
    issue serialization), pre-TileContext DMA issue (walrus codegen
    fault), 256-col exp groups (ACT overhead), merged cross-half DVE
    ops (exp1/exp2 serialization), var-first split transfers (+500ns:
    2nd-transfer issue serialization), N_DUMMY=9/10, and bf16-mode tail
    dummies for the PE mode switch (+900ns median).
  * Remaining time is framework-fixed: ~6.5us Tile/Bass entry (3 engine
    barriers + param loads), ~1.2us DMA completion-semaphore observation,
    ~2.9us epilogue barrier + NEFF teardown.
"""

import sys
import types

for _p in ("/opt/trn_rl_repo", "/opt/trn_rl_repo/concourse"):
    if _p not in sys.path:
        sys.path.insert(0, _p)

import numpy as np

import bass_rust as _bass_rust
import concourse.bacc as bacc
import concourse.bass as bass  # noqa: F401  (AP helpers)
import concourse.tile as tile
from concourse import mybir
from concourse.bass_utils import run_bass_kernel_spmd
from concourse.dve_ops import TENSOR_TENSOR_REDUCE as _TTR
from concourse.hw_specs import get_activation_tables

F32 = mybir.dt.float32
U8 = mybir.dt.uint8
F8 = mybir.dt.float8e4
F32R = mybir.dt.float32r
BF16 = mybir.dt.bfloat16
AF = mybir.ActivationFunctionType
ALU = mybir.AluOpType

N2 = 1024  # 2N rows
D = 128
TEMP = 0.01
C = 0.5 * TEMP  # 0.005
N_CORES = 8
N_DUMMY = 8  # PE warm-up matmuls (DVFS ramp) during the input DMA window

_CACHED_NC = None


def _patched_act_table_loads(self):
    """insert_act_table_loads steered so Exp and Ln resolve to the one set
    that has both (`natural_log_exp_and_others`) -> a single ACT_TABLE_LOAD
    instead of thrashing between `exp_and_others` and `natural_log` (~1.3us
    per reload).  The list ORDER must stay untouched (act_func_set_id is the
    index into act_info.json), so instead of reordering we strip Exp/Ln from
    every other set's function list."""
    has_activation = any(
        isinstance(i, mybir.InstActivation)
        for b in self.main_func.blocks
        for i in b.instructions
    )
    if not has_activation:
        return
    keep = "natural_log_exp_and_others"
    tables = [
        (name,
         funcs if name == keep
         else {f for f in funcs if f not in (AF.Exp, AF.Ln)})
        for name, funcs in get_activation_tables(self.m.arch).items()
    ]
    _bass_rust.insert_act_table_loads(self, tables)


def _recip_approx_fast(nc, out, in_):
    """reciprocal_approx_fast on arbitrary-dtype APs.  The bass wrapper
    asserts fp32 in AND out, but the DVE converts operands to fp32
    in-pipeline before the BITWISE_NOT exponent-flip seed, so bf16 input
    works; the output store rounds to the out AP's dtype."""
    from concourse.dve_ops import RECIP_APPROX_FAST_CONSTS, RECIPROCAL_APPROX_FAST

    c = RECIP_APPROX_FAST_CONSTS
    return nc.vector._custom_dve(
        RECIPROCAL_APPROX_FAST, out=out, in0=in_,
        s0=c["s0"], s1=c["s1"], imm2=c["imm2"])


def build_nc(loop_n=None):
    from contextlib import nullcontext

    nc = bacc.Bacc(None, target_bir_lowering=False, debug=False)
    nc.insert_act_table_loads = types.MethodType(_patched_act_table_loads, nc)

    # Host supplies two packed halves: [mu_half bf16 | var_half fp8e4m3]
    # = [128, 1536] bytes.
    h0_d = nc.dram_tensor("h0", [D, 1536], U8, kind="ExternalInput")
    h1_d = nc.dram_tensor("h1", [D, 1536], U8, kind="ExternalInput")
    loss_d = nc.dram_tensor("loss", [2, 1], F32, kind="ExternalOutput")

    with tile.TileContext(nc) as tc:
        with (
            tc.tile_pool(name="consts", bufs=1) as consts,
            tc.tile_pool(name="nat", bufs=1) as nat,
            tc.tile_pool(name="big", bufs=1) as big,
            tc.tile_pool(name="small", bufs=1) as small,
            tc.tile_pool(name="psum", bufs=1, space="PSUM") as psum,
        ):
            loop_cm = tc.For_i(0, loop_n, 1) if loop_n else nullcontext()
            with loop_cm:
                body(nc, tc, consts, nat, big, small, psum,
                     h0_d, h1_d, loss_d)

    nc.compile()
    return nc


def body(nc, tc, consts, nat, big, small, psum, h0_d, h1_d, loss_d):
    # ---- constants ----
    # dummy_mv memset FIRST: it is the only dependency of the PE warm-up
    # matmuls, so the DVFS ramp starts ~1.3us earlier than it would behind
    # the old ones->f32r CAST chain (the PE clock takes ~5-6us of sustained
    # activity to reach full speed; critical matmuls ran 630ns vs 376ns
    # when the ramp started late).
    dummy_mv = consts.tile([128, 512], F32)
    nc.gpsimd.memset(dummy_mv, 1.0)
    ones_f32 = consts.tile([128, 128], F32)
    nc.gpsimd.memset(ones_f32, 1.0)

    # ---- input DMA: one transfer per HWDGE queue ----
    # (Splitting each half into two issue-interleaved quarters was
    # measured SLOWER -- +0.2..1.5us -- matching the known ~0.7us issue
    # serialization per extra chunk on a queue.)
    buf = nat.tile([128, 3072], U8)
    nc.sync.dma_start(out=buf[:, 0:1536], in_=h0_d[:])
    nc.scalar.dma_start(out=buf[:, 1536:3072], in_=h1_d[:])
    mt0 = buf[:, 0:1024].bitcast(BF16)      # [128, 512] bf16
    vt0 = buf[:, 1024:1536].bitcast(F8)     # [128, 512] fp8
    mt1 = buf[:, 1536:2560].bitcast(BF16)
    vt1 = buf[:, 2560:3072].bitcast(F8)

    ones128_bf = consts.tile([128, 128], BF16)
    nc.gpsimd.memset(ones128_bf, 1.0)
    ones_col_bf = ones128_bf[:, 0:1]
    ident = consts.tile([128, 128], F32)
    # iota[p, x] = p - x ; == 0 on the diagonal
    nc.gpsimd.affine_select(
        out=ident,
        in_=ones_f32,
        pattern=[[-1, 128]],
        base=0,
        channel_multiplier=1,
        compare_op=ALU.is_equal,
        fill=0.0,
    )
    cd_bias = consts.tile([128, 1], F32)
    nc.gpsimd.memset(cd_bias, float(C * D))
    # ACT warm-up: trigger the (single) exp+ln table load at t~0 so it
    # overlaps the input DMA instead of stalling the first real Ln.
    warm = consts.tile([128, 1], F32)
    nc.scalar.activation(warm, ones_f32[:, 0:1], AF.Ln)

    # ---- PSUM ----
    p_R1 = psum.tile([128, 512], F32)
    p_R2 = psum.tile([128, 512], F32)
    p_L = psum.tile([128, 1], F32)
    p_loss = psum.tile([2, 1], F32)
    p_dummy = psum.tile([128, 512], F32)

    # ---- PE warm-up: ramp the tensor-engine clock during the DMA wait ----
    for _ in range(N_DUMMY):
        nc.tensor.matmul(p_dummy, dummy_mv[:, 0:128].bitcast(F32R),
                         dummy_mv.bitcast(F32R), start=True, stop=True)

    # ---- derived per-column tensors (j-side, bf16) ----
    lv = big.tile([128, N2], BF16)
    iv = big.tile([128, N2], BF16)
    muiv = big.tile([128, N2], BF16)  # -2 * mu * iv
    h1 = big.tile([128, N2], BF16)    # mu^2 * iv

    nc.scalar.activation(lv[:, 0:512], vt0, AF.Ln)
    nc.scalar.activation(lv[:, 512:1024], vt1, AF.Ln)

    # own-block stationaries (cols 0:128 = own rows, [d, i] layout).
    # Scalar foldings (-2, mu^2) live on these tiny [128,128] ops so the
    # full-width [128,512] muls stay plain tensor_tensor at the 2x bf16
    # rate (scalar_tensor_tensor measured 1x: ~680ns vs ~417ns).
    # sq/a_own run on GPSIMD tensor_tensor (~430ns there) to shorten the
    # critical DVE queue (A/B: ~300ns better than DVE).
    sq_own = small.tile([128, 128], BF16)
    nc.gpsimd.tensor_mul(sq_own, mt0[:, 0:128], mt0[:, 0:128])
    a_own = small.tile([128, 128], BF16)  # (mu^2 + var) own block
    nc.gpsimd.tensor_add(a_own, vt0[:, 0:128], sq_own)

    # DVE chains per column half, in data-arrival order.  The
    # tile_wait_until stamps pin the DVE queue order: the scheduler's sim
    # otherwise hoists recip1 between recip0 and muiv0 (its DMA model
    # expects half 1 early), which delays h10 -> R1-close -> exp1 by
    # ~700ns on hardware.
    with tc.tile_wait_until(0.010):
        mu2_own = small.tile([128, 128], BF16)  # -2 * mu own block
        nc.vector.tensor_scalar_mul(mu2_own, mt0[:, 0:128], -2.0)
    with tc.tile_wait_until(0.011):
        _recip_approx_fast(nc, out=iv[:, 0:512], in_=vt0)
    with tc.tile_wait_until(0.012):
        nc.vector.tensor_mul(muiv[:, 0:512], mt0, iv[:, 0:512])
    with tc.tile_wait_until(0.013):
        nc.vector.tensor_mul(h1[:, 0:512], muiv[:, 0:512], mt0)
    with tc.tile_wait_until(0.014):
        _recip_approx_fast(nc, out=iv[:, 512:1024], in_=vt1)
    with tc.tile_wait_until(0.015):
        nc.vector.tensor_mul(muiv[:, 512:1024], mt1, iv[:, 512:1024])
    with tc.tile_wait_until(0.016):
        nc.vector.tensor_mul(h1[:, 512:1024], muiv[:, 512:1024], mt1)

    # ---- main matmuls: R accumulated in PSUM ----
    sumexp_c = small.tile([128, 3], F32)

    nc.tensor.matmul(p_R1, ones128_bf, lv[:, 0:512], start=True, stop=False)
    nc.tensor.matmul(p_R1, a_own, iv[:, 0:512], start=False, stop=False)
    nc.tensor.matmul(p_R1, mu2_own, muiv[:, 0:512], start=False, stop=False)
    nc.tensor.matmul(p_R1, ones128_bf, h1[:, 0:512], start=False, stop=True)

    # L_own[i] = sum_d lv[d, i] over the own columns.
    nc.tensor.matmul(p_L, lv[:, 0:128], ones_col_bf, start=True, stop=True)

    exp_scr = big.tile([128, 512], BF16)
    nc.scalar.activation(exp_scr, p_R1, AF.Exp, scale=C,
                         accum_out=sumexp_c[:, 0:1])
    diag_exp = small.tile([128, 1], F32)
    nc.scalar.activation(diag_exp, p_L, AF.Exp, scale=C, bias=cd_bias)

    nc.tensor.matmul(p_R2, ones128_bf, lv[:, 512:1024], start=True, stop=False)
    nc.tensor.matmul(p_R2, a_own, iv[:, 512:1024], start=False, stop=False)
    nc.tensor.matmul(p_R2, mu2_own, muiv[:, 512:1024], start=False, stop=False)
    nc.tensor.matmul(p_R2, ones128_bf, h1[:, 512:1024], start=False, stop=True)

    # positive-pair extraction: diag of R1[:, 128:256], pre-scaled by C,
    # in ONE custom-DVE tensor_tensor_reduce (mask-mul + row-sum;
    # body = in0*in1*s1, accum=add with init s0).
    res2 = small.tile([128, 2], F32)  # [C*pos | ln_s] -> one stationary
    pos_scr = small.tile([128, 128], F32)
    with tc.tile_wait_until(0.017):
        nc.vector._custom_dve(
            _TTR, out=pos_scr, in0=p_R1[:, 128:256], in1=ident,
            s0=0.0, s1=float(C), accum_out=res2[:, 0:1])

    exp_scr2 = big.tile([128, 512], BF16)
    nc.scalar.activation(exp_scr2, p_R2, AF.Exp, scale=C,
                         accum_out=sumexp_c[:, 1:2])

    # ln(sum_j exp) with the diagonal removed: pre = group1 - self_exp is
    # computed as soon as the first accumulator lands, then folded into the
    # final Ln as its per-partition bias: ln_s = Ln(group2 + pre).
    pre_adj = small.tile([128, 1], F32)
    with tc.tile_wait_until(0.018):
        nc.vector.tensor_sub(pre_adj, sumexp_c[:, 0:1], diag_exp)
    nc.scalar.activation(res2[:, 1:2], sumexp_c[:, 1:2], AF.Ln, bias=pre_adj)

    # Contract the 128 partitions with a ones column -> [2,1] per-core
    # [sum(C*pos), sum(ln_s)]; the host takes the difference and /2N.
    nc.tensor.matmul(p_loss, res2, ones_f32[:, 0:1], start=True, stop=True)
    loss_row = small.tile([2, 1], F32)
    with tc.tile_wait_until(0.019):
        nc.vector.tensor_copy(loss_row, p_loss)
    nc.sync.dma_start(out=loss_d[:], in_=loss_row, single_packet=True)


# Per-core column permutation: [own 0:128 | pos block | remaining].
_P = np.concatenate([np.arange(0, 128), np.arange(512, 1024),
                     np.arange(128, 512)]).astype(np.int64)


def run_spmd(p1_loc, p2_loc, p1_scale, p2_scale, **spmd_kwargs):
    """Shard, run on 8 cores, gather.  Returns (loss_scalar, BassKernelResults)."""
    global _CACHED_NC
    import ml_dtypes
    mu_t = np.concatenate([p1_loc, p2_loc], axis=0).astype(np.float32).T
    var_t = np.concatenate([p1_scale, p2_scale], axis=0).astype(np.float32).T
    mu_t = np.ascontiguousarray(mu_t).astype(ml_dtypes.bfloat16)   # [D, 2N]
    var_t = np.ascontiguousarray(var_t).astype(ml_dtypes.float8_e4m3fn)
    if _CACHED_NC is None:
        _CACHED_NC = build_nc()
    nc = _CACHED_NC
    in_maps = []
    for c in range(N_CORES):
        cols = (_P + 128 * c) % N2
        mu_c = mu_t[:, cols]
        var_c = var_t[:, cols]
        def pack(mu_h, var_h):
            return np.ascontiguousarray(np.concatenate(
                [np.ascontiguousarray(mu_h).view(np.uint8),
                 np.ascontiguousarray(var_h).view(np.uint8)], axis=1))
        in_maps.append({
            "h0": pack(mu_c[:, 0:512], var_c[:, 0:512]),
            "h1": pack(mu_c[:, 512:1024], var_c[:, 512:1024]),
        })
    res = run_bass_kernel_spmd(nc, in_maps, core_ids=list(range(N_CORES)),
                               **spmd_kwargs)
    acc = np.sum([r["loss"].reshape(2) for r in res.results], axis=0,
                 dtype=np.float64)
    return np.array((acc[0] - acc[1]) / N2, dtype=np.float32), res


def kernel(p1_loc, p2_loc, p1_scale, p2_scale):
    loss, _ = run_spmd(p1_loc, p2_loc, p1_scale, p2_scale)
    return loss


if __name__ == "__main__":
    import reference

    inputs = reference.setup_inputs()
    expected = np.asarray(reference.reference(**inputs))
    actual = kernel(**{k: np.asarray(v) for k, v in inputs.items()})
    rel = abs(float(actual) - float(expected)) / max(abs(float(expected)), 1e-30)
    print("expected:", expected, "actual:", actual, "rel err:", rel)
